# revision 12
# baseline (speedup 1.0000x reference)
"""Trainium2 Bass kernel for nn_MaxSel: max of 4 directional 3x3 Laplacians.

Math: for each of B*C independent single-channel images,
  out[i,j] = max(h, v, d1, d2) - 2*x[i,j]   (zero padding), where
    h  = x[i,j-1] + x[i,j+1]
    v  = x[i-1,j] + x[i+1,j]
    d1 = x[i-1,j+1] + x[i+1,j-1]
    d2 = x[i-1,j-1] + x[i+1,j+1]

Sharding: pure data parallel over batch: core k takes x[2k:2k+2] (6
channel-images of 512x512 per core).

Layout ("strip"): each SBUF partition holds F=8 consecutive image rows
plus a one-row halo above/below (10 row-slots), each row padded to 514
columns with zeroed pad columns, plus a 2-element leading pad.  All 8
stencil neighbors become free-dimension offsets; the zero pads absorb
every edge case.  One image = 64 strips; 2 images per 128-partition
tile; 3 tiles per core.
"""

import sys

sys.path.insert(0, "/opt/trn_rl_repo")

import numpy as np

import concourse.bass as bass
import concourse.mybir as mybir
from concourse.ap import AP

F32 = mybir.dt.float32
ALU = mybir.AluOpType

N_CORES = 8
B, C, H, W = 16, 3, 512, 512
BPC = B // N_CORES            # batch per core
NIMG = BPC * C                # 6 images per core
WP = W + 2                    # padded row width
FROWS = 8                     # image rows per strip
SLOTS = FROWS + 2             # row slots incl. halo
SPI = H // FROWS              # strips per image = 64
IMGS_PER_TILE = 128 // SPI    # 2
NTILES = NIMG // IMGS_PER_TILE  # 3
LEAD = 2                      # leading pad elements
TAIL = 2                      # trailing slack (read by pad-col outputs)
IN_FREE = LEAD + SLOTS * WP + TAIL  # 5144
OUT_FREE = FROWS * WP         # 4112
CLEN = FROWS * WP             # compute length per partition (= OUT_FREE)
CBASE = LEAD + WP             # free offset of first interior element
IMG = H * W                   # elements per image


def build_kernel(nc: bass.Bass, x, y):
    """Emit the per-core kernel body. x: [BPC,C,H,W] in, y: same out."""
    xf = x[:].rearrange("b c h w -> (b c) (h w)")   # [6, 262144]
    yf = y[:].rearrange("b c h w -> (b c) h w")     # [6, 512, 512]

    with (
        nc.sbuf_tensor([128, NTILES * IN_FREE], F32) as in_t,
        nc.sbuf_tensor([128, NTILES * OUT_FREE], F32) as out_t,
        nc.sbuf_tensor([128, CLEN], F32) as tmp1,
        nc.sbuf_tensor([128, CLEN], F32) as tmp2,
        nc.semaphore("mem_sem") as mem_sem,
        nc.semaphore("in_sem0") as in_sem0,
        nc.semaphore("in_sem1") as in_sem1,
        nc.semaphore("in_sem2") as in_sem2,
        nc.semaphore("cmp_sem") as cmp_sem,
        nc.semaphore("out_sem") as out_sem,
        nc.Block() as block,
    ):
        def in_tile(t):
            return in_t[:, t * IN_FREE:(t + 1) * IN_FREE]

        def out_tile(t):
            return out_t[:, t * OUT_FREE:(t + 1) * OUT_FREE]

        in_sems = [in_sem0, in_sem1, in_sem2]

        @block.gpsimd
        def _(gpsimd):
            # Zero everything the DMAs don't write but compute reads:
            # leading/trailing pad, pad columns of every slot, and the
            # top/bottom halo row-slots.  Slot 0 and slot SLOTS-1 are
            # zeroed across ALL partitions (engine ops must start at a
            # 32-aligned partition); the DMAs then overwrite the
            # non-halo partitions with real data, so these memsets must
            # complete before the DMAs start (mem_sem).
            for t in range(NTILES):
                it = in_tile(t)
                gpsimd.memset(it[:, 0:LEAD], 0.0)
                gpsimd.memset(it[:, IN_FREE - TAIL:IN_FREE], 0.0)
                pads = it[:, LEAD:LEAD + SLOTS * WP] \
                    .rearrange("p (r c) -> p r c", c=WP)[:, :, W:WP]
                gpsimd.memset(pads, 0.0)
                gpsimd.memset(it[:, LEAD:LEAD + W], 0.0)
                gpsimd.memset(
                    it[:, LEAD + (SLOTS - 1) * WP:
                       LEAD + (SLOTS - 1) * WP + W], 0.0)
            gpsimd.memset(tmp1[0:32, 0:1], 0.0).then_inc(mem_sem, 1)

        @block.sync
        def _(sync):
            sync.wait_ge(mem_sem, 1)
            for t in range(NTILES):
                it = in_tile(t)
                for k in range(IMGS_PER_TILE):
                    img = t * IMGS_PER_TILE + k
                    p0 = k * SPI
                    # main: strips 1..62, 10 overlapping row-slots each
                    src = AP(
                        xf.tensor,
                        img * IMG + (FROWS - 1) * W,
                        [[FROWS * W, SPI - 2], [W, SLOTS], [1, W]],
                    )
                    dst = it[p0 + 1:p0 + SPI - 1, LEAD:LEAD + SLOTS * WP] \
                        .rearrange("p (r c) -> p r c", c=WP)[:, :, 0:W]
                    sync.dma_start(dst, src).then_inc(in_sems[t], 16)
                    # top strip 0: rows 0..8 into slots 1..9
                    src = xf[img][0:(SLOTS - 1) * W] \
                        .rearrange("(r c) -> r c", c=W)
                    dst = it[p0:p0 + 1, LEAD + WP:LEAD + SLOTS * WP] \
                        .rearrange("p (r c) -> p r c", c=WP)[:, :, 0:W]
                    sync.dma_start(dst, src.unsqueeze(0)).then_inc(in_sems[t], 16)
                    # bottom strip 63: rows 503..511 into slots 0..8
                    src = xf[img][(H - SLOTS + 1) * W:] \
                        .rearrange("(r c) -> r c", c=W)
                    dst = it[p0 + SPI - 1:p0 + SPI,
                             LEAD:LEAD + (SLOTS - 1) * WP] \
                        .rearrange("p (r c) -> p r c", c=WP)[:, :, 0:W]
                    sync.dma_start(dst, src.unsqueeze(0)).then_inc(in_sems[t], 16)
            # outputs
            for t in range(NTILES):
                sync.wait_ge(cmp_sem, t + 1)
                ot = out_tile(t)
                for k in range(IMGS_PER_TILE):
                    img = t * IMGS_PER_TILE + k
                    p0 = k * SPI
                    src = ot[p0:p0 + SPI, :] \
                        .rearrange("p (r c) -> p r c", c=WP)[:, :, 0:W]
                    dst = yf[img].rearrange("(s r) c -> s r c", r=FROWS)
                    sync.dma_start(dst, src).then_inc(out_sem, 16)

        @block.vector
        def _(vector):
            for t in range(NTILES):
                vector.wait_ge(in_sems[t], 16 * 3 * IMGS_PER_TILE)
                it = in_tile(t)
                ot = out_tile(t)

                def c(d):
                    return it[:, CBASE + d:CBASE + d + CLEN]

                nc.vector.tensor_tensor(tmp1[:], c(-1), c(1), op=ALU.add)
                nc.vector.tensor_tensor(tmp2[:], c(-WP), c(WP), op=ALU.add)
                nc.vector.tensor_tensor(tmp1[:], tmp1[:], tmp2[:], op=ALU.max)
                nc.vector.tensor_tensor(tmp2[:], c(-(WP - 1)), c(WP - 1),
                                        op=ALU.add)
                nc.vector.tensor_tensor(ot[:], c(-(WP + 1)), c(WP + 1),
                                        op=ALU.add)
                nc.vector.tensor_tensor(tmp2[:], tmp2[:], ot[:], op=ALU.max)
                nc.vector.tensor_tensor(tmp1[:], tmp1[:], tmp2[:], op=ALU.max)
                nc.vector.scalar_tensor_tensor(
                    ot[:], c(0), -2.0, tmp1[:],
                    op0=ALU.mult, op1=ALU.add).then_inc(cmp_sem, 1)

    return nc


def make_bass():
    nc = bass.Bass()
    x = nc.declare_dram_parameter("x", [BPC, C, H, W], F32, isOutput=False)
    y = nc.declare_dram_parameter("y", [BPC, C, H, W], F32, isOutput=True)
    build_kernel(nc, x, y)
    return nc


TRACE = False          # set by test.py to capture an NTFF profile
LAST_RESULTS = None    # BassKernelResults of the last kernel() call


def kernel(x):
    global LAST_RESULTS
    from concourse.bass_utils import run_bass_kernel_spmd

    x = np.ascontiguousarray(np.asarray(x, dtype=np.float32))
    nc = make_bass()
    in_maps = [{"x": x[i * BPC:(i + 1) * BPC]} for i in range(N_CORES)]
    res = run_bass_kernel_spmd(nc, in_maps, list(range(N_CORES)), trace=TRACE)
    LAST_RESULTS = res
    return np.concatenate([res.results[i]["y"] for i in range(N_CORES)], axis=0)
